# revision 12
# baseline (speedup 1.0000x reference)
"""DiffusionGraphConv Trainium2 kernel (bf16 operands, f32 accumulation).

Math (per batch b, support s, A = supports[s]):
  x0 = concat(inputs, state)                      # [N, F=128]
  out = sum_k x_k @ W_k (+bias), k in {x0, x1_s0, x2_s0, x1_s1, x2_s1}
  with x1 = A x0, x2 = 2 A A x0 - x0.

Restructured (no on-chip transposes, minimal staging):
  out = x0 @ What + bias + sum_s A_s @ (x0 @ W1_s + A_s @ (x0 @ (2*W2_s)))
  with What = W_0 - W_2 - W_4.

Only u_s = x0 @ (2*W2_s) is staged through SBUF (both supports in one MM
per (b, node-chunk)).  The x0@W1_s and x0@What products are folded into the
A-multiply PSUM accumulation chains as extra 128-wide matmuls, and the bias
is folded into the fin0 PSUM-evacuation add.  All operands are bf16 (host
casts); PSUM accumulates fp32; per-term rel-err ~4e-3 vs the 2e-2 gate.

Layouts (per core, batch-sharded B_local = 8):
  x0T DRAM [b=8, F=128, m=1024] bf16, A^T DRAM [s=2, m, n] bf16 (host-staged)
  out DRAM [h=2, n=1024, 4, O] bf16 — contiguous 128KB per bank DMA.
"""

import sys as _sys
import types as _types

try:
    import antenv.axon_hooks  # noqa: F401
except Exception:
    try:
        import antenv as _antenv

        _m = _types.ModuleType("antenv.axon_hooks")
        _m._hook = None
        _m.set_axon_ntff_profile_hook = lambda h: setattr(_m, "_hook", h)
        _m.get_axon_ntff_profile_hook = lambda: _m._hook
        _sys.modules["antenv.axon_hooks"] = _m
        _antenv.axon_hooks = _m
    except Exception:
        pass

import ml_dtypes
import numpy as np

import concourse.mybir as mybir
import concourse.tile as tile
from concourse import bacc
from concourse.bass_utils import run_bass_kernel_spmd

NCORES = 8
B = 64
BL = B // NCORES  # 8 batches per core
N = 1024
F = 128
O = 128
NCH = N // 128  # 8 node chunks

BF16 = mybir.dt.bfloat16
F32 = mybir.dt.float32

_CACHE = {}


def _build():
    if "nc" in _CACHE:
        return _CACHE["nc"]

    nc = bacc.Bacc(trn_type="TRN2", num_devices=NCORES, debug=False)

    # chunked host layouts: 4 chunks per tensor, partition-major inside a
    # chunk so each DMA reads contiguous 4KB per partition (2 batches /
    # 2 node-rows of A^T per chunk)
    x0t_d = nc.dram_tensor("x0t", [4, F, 2, N], BF16, kind="ExternalInput")
    at_d = nc.dram_tensor("at", [2, 4, 128, 2, N], BF16, kind="ExternalInput")
    # w slots host-prepped: [What=W0-W2-W4, 2*W2, 2*W4, W1, W3]
    w_d = nc.dram_tensor("w", [F * 5, O], BF16, kind="ExternalInput")
    b_d = nc.dram_tensor("b", [1, BL * O], F32, kind="ExternalInput")
    out_d = nc.dram_tensor("out", [2, N, 4, O], BF16, kind="ExternalOutput")

    with tile.TileContext(nc) as tc:
        with (
            tc.tile_pool(name="big", bufs=1) as big,
            tc.tile_pool(name="small", bufs=1) as small,
            tc.tile_pool(name="outp", bufs=4) as outp,
            tc.tile_pool(name="ps_pool", bufs=8, space="PSUM") as ps_pool,
        ):
            # ---- persistent tiles ----
            wc = small.tile([F, 5, O], BF16)
            bias_t = small.tile([1, BL * O], F32)
            b1024 = small.tile([128, BL * O], F32)
            x0t_t = big.tile([F, BL, N], BF16, name="x0t_t")
            at_t0 = big.tile([128, NCH, N], BF16, name="at_t0")
            at_t1 = big.tile([128, NCH, N], BF16, name="at_t1")
            # u[:, mi, b, s*128+o] = (x0 @ 2W2_s)[node, o] for node chunk mi
            u_t = big.tile([128, NCH, BL, 256], BF16, name="u_t")
            v0 = big.tile([128, NCH, N], BF16, name="v0")
            v1 = big.tile([128, NCH, N], BF16, name="v1")
            # fins[:, ni, b*128+o] f32: x0@What + bias + A0-chain terms
            fins = big.tile([128, NCH, BL * O], F32, name="fins")

            # ---- PE warm-up during the DMA head (HAM un-throttle).
            # dummy is a raw (untracked) SBUF tensor — garbage contents are
            # fine, and no memset dependency delays the first matmul.
            dummy = nc.alloc_sbuf_tensor("dummy_warm", [128, 256], BF16).ap()
            dsink = small.tile([128, 1], F32)
            for _ in range(14):
                pw = ps_pool.tile([128, 512], F32, name="ps_w", tag="ps")
                nc.tensor.matmul(
                    pw[:, 0:256], dummy[:, 0:128], dummy[:], start=True, stop=True
                )
            nc.vector.tensor_copy(dsink[:], pw[:, 0:1])

            # ---- input DMAs (sync FIFO: wc -> x0t -> at0 -> at1),
            # 512KB chunks: issue cost (~0.6us) < transfer (~0.72us)
            nc.sync.dma_start(wc[:], w_d.rearrange("(f k) o -> f k o", k=5))
            nc.scalar.dma_start(bias_t[:], b_d[:])
            for k in range(4):
                nc.sync.dma_start(x0t_t[:, 2 * k : 2 * k + 2, :], x0t_d[k])
            for k in range(4):
                nc.sync.dma_start(at_t0[:, 2 * k : 2 * k + 2, :], at_d[0, k])
            for k in range(4):
                nc.sync.dma_start(at_t1[:, 2 * k : 2 * k + 2, :], at_d[1, k])

            nc.gpsimd.partition_broadcast(b1024[:], bias_t[:])

            cnt = [0]

            def evac(dst, src):
                # alternate PSUM evacuation between DVE and ACT
                cnt[0] += 1
                if cnt[0] % 2 == 0:
                    nc.vector.tensor_copy(dst, src)
                else:
                    nc.scalar.copy(dst, src)

            # ---- Sa step (b, mi): u for both supports in one 256-wide MM
            def sa_step(b, mi):
                ps = ps_pool.tile([128, 512], F32, name="ps_sa", tag="ps")
                nc.tensor.matmul(
                    ps[:, 0:256],
                    x0t_t[:, b, mi * 128 : (mi + 1) * 128],
                    wc[:, 1:3, :],
                    start=True,
                    stop=True,
                )
                evac(u_t[:, mi, b, :], ps[:, 0:256])

            # ---- v bank (s, ni, h): v_s = A_s @ u_s + x0 @ W1_s
            def v_bank(s, at_t, v, ni, h):
                pv = ps_pool.tile([128, 512], F32, name="ps_v", tag="ps")
                for j in range(4):
                    nc.tensor.matmul(
                        pv[:, j * 128 : (j + 1) * 128],
                        x0t_t[:, 4 * h + j, ni * 128 : (ni + 1) * 128],
                        wc[:, 3 + s, :],
                        start=(j == 0),
                        stop=False,
                    )
                for mi in range(NCH):
                    nc.tensor.matmul(
                        pv[:],
                        at_t[:, mi, ni * 128 : (ni + 1) * 128],
                        u_t[:, mi, 4 * h : 4 * h + 4, s * 128 : (s + 1) * 128],
                        start=False,
                        stop=(mi == NCH - 1),
                    )
                evac(v[:, ni, h * 512 : (h + 1) * 512], pv[:])

            # ---- fin0 bank (ni, h): fins = x0@What + bias + A0 @ v0
            def fin0_bank(ni, h):
                pf = ps_pool.tile([128, 512], F32, name="ps_f", tag="ps")
                for j in range(4):
                    nc.tensor.matmul(
                        pf[:, j * 128 : (j + 1) * 128],
                        x0t_t[:, 4 * h + j, ni * 128 : (ni + 1) * 128],
                        wc[:, 0, :],
                        start=(j == 0),
                        stop=False,
                    )
                for mi in range(NCH):
                    nc.tensor.matmul(
                        pf[:],
                        at_t0[:, mi, ni * 128 : (ni + 1) * 128],
                        v0[:, mi, h * 512 : (h + 1) * 512],
                        start=False,
                        stop=(mi == NCH - 1),
                    )
                fslc = fins[:, ni, h * 512 : (h + 1) * 512]
                nc.vector.tensor_add(
                    fslc, pf[:], b1024[:, h * 512 : (h + 1) * 512]
                )

            # ---- fin1 bank (ni, h): out = fins + A1 @ v1 -> bf16 -> DRAM
            # last=True splits the evac+DMA in halves to shorten the tail
            def fin1_bank(ni, h, last=False):
                pf = ps_pool.tile([128, 512], F32, name="ps_f", tag="ps")
                for mi in range(NCH):
                    nc.tensor.matmul(
                        pf[:],
                        at_t1[:, mi, ni * 128 : (ni + 1) * 128],
                        v1[:, mi, h * 512 : (h + 1) * 512],
                        start=(mi == 0),
                        stop=(mi == NCH - 1),
                    )
                ot = outp.tile([128, 512], BF16, name="ot", tag="ot")
                fslc = fins[:, ni, h * 512 : (h + 1) * 512]
                dslc = out_d[h, ni * 128 : (ni + 1) * 128, :, :]
                if not last:
                    nc.vector.tensor_add(ot[:], pf[:], fslc)
                    nc.sync.dma_start(dslc, ot[:])
                else:
                    for c in range(2):
                        sl = slice(c * 256, (c + 1) * 256)
                        nc.vector.tensor_add(ot[:, sl], pf[:, sl], fslc[:, sl])
                        nc.sync.dma_start(
                            out_d[
                                h,
                                ni * 128 : (ni + 1) * 128,
                                2 * c : 2 * c + 2,
                                :,
                            ],
                            ot[:, sl],
                        )

            # ---- schedule (emission order ~= PE execution order) ----
            # Sa for batches 0-3, then v0 h=0 banks interleaved with Sa 4-7
            for b in range(4):
                for mi in range(NCH):
                    sa_step(b, mi)
            for ni in range(NCH):
                v_bank(0, at_t0, v0, ni, 0)
                if ni < 4:
                    for mi in range(NCH):
                        sa_step(4 + ni, mi)
            for ni in range(NCH):
                v_bank(0, at_t0, v0, ni, 1)
            for ni in range(NCH):
                fin0_bank(ni, 0)
            for ni in range(NCH):
                fin0_bank(ni, 1)
            for ni in range(NCH):
                v_bank(1, at_t1, v1, ni, 0)
            for ni in range(NCH):
                v_bank(1, at_t1, v1, ni, 1)
            for ni in range(NCH):
                fin1_bank(ni, 0)
            for ni in range(NCH):
                fin1_bank(ni, 1, last=(ni == NCH - 1))

    nc.compile()
    _CACHE["nc"] = nc
    return nc


def kernel(supports, inputs, state, weight, biases, output_size, _trace=False):
    supports = np.asarray(supports, dtype=np.float32)
    inputs = np.asarray(inputs, dtype=np.float32)
    state = np.asarray(state, dtype=np.float32)
    weight = np.asarray(weight, dtype=np.float32)
    biases = np.asarray(biases, dtype=np.float32)
    O_ = int(output_size)
    assert O_ == O and inputs.shape == (B, N * 64) and supports.shape == (2, N, N)

    nc = _build()

    bf = ml_dtypes.bfloat16
    # host staging (layout + cast): A^T, x0^T bf16, DMA-chunked
    # (4 chunks/tensor, partition-major inside each chunk);
    # W slots prepped in f32: [What=W0-W2-W4, 2*W2, 2*W4, W1, W3]
    at_t = supports.transpose(0, 2, 1)  # [2, m, n]
    # [2, 4, 128, 2, n]: chunk k = A^T rows 256k..256k+256, m = 256k+128c+p
    at_np = np.ascontiguousarray(
        at_t.reshape(2, 4, 2, 128, N).transpose(0, 1, 3, 2, 4)
    ).astype(bf)
    x0 = np.concatenate(
        [inputs.reshape(B, N, 64), state.reshape(B, N, 64)], axis=2
    )  # [B, N, F]
    x0t = x0.transpose(0, 2, 1)  # [B, F, N] view
    wg = weight.reshape(F, 5, O)
    wk = np.ascontiguousarray(
        np.stack(
            [
                wg[:, 0] - wg[:, 2] - wg[:, 4],
                2.0 * wg[:, 2],
                2.0 * wg[:, 4],
                wg[:, 1],
                wg[:, 3],
            ],
            axis=1,
        )
    ).reshape(F * 5, O).astype(bf)
    brow = np.ascontiguousarray(np.tile(biases, BL)[None, :]).astype(np.float32)

    in_maps = []
    for c in range(NCORES):
        xc = x0t[c * BL : (c + 1) * BL]  # [8, F, N]
        xc = np.ascontiguousarray(
            xc.reshape(4, 2, F, N).transpose(0, 2, 1, 3)
        ).astype(bf)  # [4, F, 2, N]
        in_maps.append({"x0t": xc, "at": at_np, "w": wk, "b": brow})

    res = run_bass_kernel_spmd(
        nc, in_maps, core_ids=list(range(NCORES)), trace=_trace
    )
    kernel.last_result = res

    # out per core: [2, N, 4, O] bf16, b_local = 4h + bb -> full [B, N*O]
    full = np.empty((B, N, O), dtype=np.float32)
    for c in range(NCORES):
        arr = np.asarray(res.results[c]["out"])  # [2, N, 4, O]
        full[c * BL : (c + 1) * BL] = (
            arr.transpose(0, 2, 1, 3).reshape(BL, N, O).astype(np.float32)
        )
    return np.ascontiguousarray(full).reshape(B, N * O_)
